# revision 12
# baseline (speedup 1.0000x reference)
"""Multi-head attention (B=2, S=2048, D=1024, H=16) on 8 Trainium2 NeuronCores.

Sharding: core c handles batch b = c//4 and head group g = c%4 (4 heads, 256
of the 1024 model dims). All matmul operands are bf16 (host pre-casts X and
weights); PSUM accumulation stays fp32.

Per core:
  kT/qT = (X @ W_{K,Q}[:, g])^T  [128, 2048] bf16 (score scale folded into
          W_Q/b_Q on host). q stored per-head zero-padded to K=128 so the
          scores matmul drives the full PE array with uniform full-array
          LDWEIGHTS pipelining.
  v     =  X @ W_V[:, g] stored [k, i, head, 128] bf16 with ALL-ONES cols
          0:64 and value cols 64:128, so each av matmul yields the softmax
          denominator replicated on psum parts 0:64 (base-0 for the custom
          approx reciprocal) AND attn@v on parts 64:128 (TT ops may read
          shifted PSUM operands).
  exp on ScalarE (no max-subtraction; scores are O(1) by construction).
  attention runs one HEAD at a time (not pair-interleaved): only one av
          psum accumulator is held across an i-loop, so the next head's
          score tiles always have free PSUM banks and the drain latency
          never stalls the PE at head boundaries.
  drain = single-pass approx reciprocal + per-head multiply writing bf16
          straight into the A2A send tile.
Combine: Q columns host-permuted so attention chunk j covers half of every
core's output rows; 8-core AllToAlls ship each chunk as soon as it drains:
per head-PAIR for chunks (0,p0) (0,p1) (1,p0), and per HEAD for the final
pair so the tail-critical last collective is quarter-size and the
second-to-last ships ~20us earlier. a2l reloads are single batched DMAs on
the sync HWDGE queue right after each collective lands; the msel
batch-select merges are emitted right after the NEXT drain so the in-order
DVE never blocks PSUM release (they run in DVE idle windows).
Output projection: full-width bf16, p-major over A2A chunks so everything
not depending on the last collective completes during its flight. Host
unpermutes.
"""

import sys

if "/opt/trn_rl_repo" not in sys.path:
    sys.path.insert(0, "/opt/trn_rl_repo")

import ml_dtypes
import numpy as np

import concourse.bass as bass
import concourse.mybir as mybir
import concourse.tile as tile
from concourse import bacc
from concourse.bass_utils import run_bass_kernel_spmd

B, S, D = 2, 2048, 1024
H, DK = 16, 64
N_CORES = 8
HPC = 4  # heads per core
EC = HPC * DK  # 256 local model dims per core
F32 = mybir.dt.float32
BF16 = mybir.dt.bfloat16

NJ = 2  # q-chunks of 1024
JW = S // NJ
NI = S // 128  # k-tiles
NP = HPC // 2  # head pairs

# q-column permutation: perm-block r (256 wide) of chunk j = global rows
# [r*512 + j*256 : r*512 + (j+1)*256], so A2A slot r always carries the rows
# core r outputs, half per j-chunk.
_PERM = np.concatenate(
    [np.arange(r * 512 + j * 256, r * 512 + (j + 1) * 256) for j in range(2) for r in range(4)]
)


def _wlayout(w):
    """[1024, EC] -> [128, 8, EC] bf16 matching the SBUF lhsT tile layout."""
    return np.ascontiguousarray(w.reshape(8, 128, EC).transpose(1, 0, 2)).astype(
        ml_dtypes.bfloat16
    )


def _wo_global(W_O):
    """[128, 8, D]: global W_O row-chunks (same for every core)."""
    out = W_O.reshape(8, 128, D).transpose(1, 0, 2)
    return np.ascontiguousarray(out).astype(ml_dtypes.bfloat16)


def _build_nc():
    nc = bacc.Bacc(None, num_devices=N_CORES, num_swdge_queues=4)

    xqt = nc.dram_tensor("xqt", [D, S], BF16, kind="ExternalInput")
    xkt = nc.dram_tensor("xkt", [D, S], BF16, kind="ExternalInput")
    xvt = nc.dram_tensor("xvt", [D, S], BF16, kind="ExternalInput")
    wq = nc.dram_tensor("wq", [128, 8, EC], BF16, kind="ExternalInput")
    wk = nc.dram_tensor("wk", [128, 8, EC], BF16, kind="ExternalInput")
    wv = nc.dram_tensor("wv", [128, 8, EC], BF16, kind="ExternalInput")
    wo = nc.dram_tensor("wo", [128, 8, D], BF16, kind="ExternalInput")
    msel = nc.dram_tensor("msel", [128, 2], F32, kind="ExternalInput")
    bq = nc.dram_tensor("bq", [EC], F32, kind="ExternalInput")
    bk = nc.dram_tensor("bk", [EC], F32, kind="ExternalInput")
    bv = nc.dram_tensor("bv", [EC], F32, kind="ExternalInput")
    bo = nc.dram_tensor("bo", [D], F32, kind="ExternalInput")

    # A2A buffers: pair-sized for (0,0) (0,1) (1,0); head-sized for the
    # final pair's two heads (tail-critical, ship separately).
    a2a_in = {
        (j, p): nc.dram_tensor(f"a2a_in{j}_{p}", [N_CORES, 128, 256], BF16)
        for (j, p) in [(0, 0), (0, 1), (1, 0)]
    }
    a2a_out = {
        (j, p): nc.dram_tensor(f"a2a_out{j}_{p}", [N_CORES, 128, 256], BF16)
        for (j, p) in [(0, 0), (0, 1), (1, 0)]
    }
    # one backing tensor for the two per-head collectives of the final
    # pair, so the a2l reload can pull BOTH heads' slots as a single
    # full-128-partition DMA (partition-offset SBUF DMA writes are NOT
    # dependency-tracked and race with their readers)
    a2a_in_h = {
        h: nc.dram_tensor(f"a2a_inh{h}", [N_CORES, DK, 256], BF16) for h in (2, 3)
    }
    a2a_out_h = {
        h: nc.dram_tensor(f"a2a_outh{h}", [N_CORES, DK, 256], BF16) for h in (2, 3)
    }
    out = nc.dram_tensor("out", [NJ, 256, D], F32, kind="ExternalOutput")

    with tile.TileContext(nc) as tc:
        with (
            tc.tile_pool(name="res", bufs=1) as res,
            tc.tile_pool(name="xt", bufs=10) as xt_pool,
            tc.tile_pool(name="exp", bufs=6) as exp_pool,
            tc.tile_pool(name="osb", bufs=3) as osb_pool,
            tc.tile_pool(name="rec", bufs=4) as rec_pool,
            tc.tile_pool(name="snd", bufs=2) as snd_pool,
            tc.tile_pool(name="a2l", bufs=16) as a2l_pool,
            tc.tile_pool(name="ps", bufs=1, space="PSUM") as ps,
        ):
            # --- weights / biases on the ACT hwdge queue, consumption order ---
            wq_sb = res.tile([128, 8, EC], BF16, tag="wq")
            wk_sb = res.tile([128, 8, EC], BF16, tag="wk")
            wv_sb = res.tile([128, 8, EC], BF16, tag="wv")
            wo_sb = res.tile([128, 8, D], BF16, tag="wo")
            msel_sb = res.tile([128, 2], F32, tag="msel")
            bq_sb = res.tile([128, 2], F32, tag="bq")
            bk_sb = res.tile([128, 2], F32, tag="bk")
            bv_rep = res.tile([128, EC], F32, tag="bv")
            bo_rep = res.tile([128, D], F32, tag="bo")
            nc.scalar.dma_start(out=wk_sb, in_=wk[:])
            nc.scalar.dma_start(out=bk_sb, in_=bk[:].rearrange("(c p) -> p c", p=128))
            nc.scalar.dma_start(out=wv_sb, in_=wv[:])
            nc.scalar.dma_start(out=wq_sb, in_=wq[:])
            nc.scalar.dma_start(out=bq_sb, in_=bq[:].rearrange("(c p) -> p c", p=128))
            nc.scalar.dma_start(out=msel_sb, in_=msel[:])
            nc.scalar.dma_start(
                out=bv_rep,
                in_=bass.AP(tensor=bv[:].tensor, offset=0, ap=[[0, 128], [1, EC]]),
            )

            # --- HAM warm-up: ~3.4us of dummy matmuls so the PE clock
            # gate opens before the first projection data lands ---
            warm = res.tile([128, 512], BF16, tag="warm")
            nc.gpsimd.memset(warm, 0.0)
            pw = ps.tile([128, 512], F32, tag="q4", bufs=4, name="pw")
            for w in range(11):
                nc.tensor.matmul(
                    pw, warm[:, 0:128], warm, start=(w == 0), stop=(w == 10)
                )

            # --- residents ---
            kt = [res.tile([128, S], BF16, tag=f"kt{c}", name=f"kt{c}") for c in range(2)]
            # per-head q, zero-padded in the complementary 64 partitions so the
            # scores matmul contracts K=128 (uniform full-array matmuls)
            qtz = [
                res.tile([128, S], BF16, tag=f"qtz{h}", name=f"qtz{h}")
                for h in range(HPC)
            ]
            for h in range(HPC):
                z = slice(64, 128) if h % 2 == 0 else slice(0, 64)
                nc.gpsimd.memset(qtz[h][z, :], 0.0)
            # v: ones cols 0:64 (denominator), value cols 64:128
            v_sb = res.tile([128, NI, HPC, 2 * DK], BF16, tag="v")
            nc.gpsimd.memset(v_sb[:, :, :, 0:DK], 1.0)

            # --- projections, interleaved k -> v-pass0 -> q -> v-pass1 so
            # the v matmuls fill the PE while xq is still streaming in ---
            # out[e, s] accumulated over d; lhsT = W d-chunk, rhs = X^T.
            def emit_kq(xsrc, w_sb, b_sb, dst):
                pk = [
                    ps.tile([128, 1024], F32, tag="q4", bufs=4, name=f"pk{_c}")
                    for _c in range(4)
                ]
                for d2 in range(4):
                    xtile = xt_pool.tile([128, 2, S], BF16, tag="xt", bufs=5)
                    nc.sync.dma_start(
                        out=xtile,
                        in_=xsrc[d2 * 256 : (d2 + 1) * 256, :].rearrange(
                            "(dd p) s -> p dd s", p=128
                        ),
                    )
                    for dd in range(2):
                        d = d2 * 2 + dd
                        for half in range(2):
                            for c in range(2):
                                for n in range(2):
                                    nc.tensor.matmul(
                                        pk[2 * half + c][:, n * 512 : (n + 1) * 512],
                                        w_sb[:, d, c * 128 : (c + 1) * 128],
                                        xtile[
                                            :,
                                            dd,
                                            half * 1024 + n * 512 : half * 1024
                                            + (n + 1) * 512,
                                        ],
                                        start=(d == 0),
                                        stop=(d == 7),
                                    )
                for half in range(2):
                    hs2 = slice(half * 1024, (half + 1) * 1024)
                    for c in range(2):
                        if dst is not None:
                            nc.vector.tensor_scalar_add(
                                dst[c][:, hs2], pk[2 * half + c], b_sb[:, c : c + 1]
                            )
                        else:
                            nc.vector.tensor_scalar_add(
                                qtz[2 * c][0:64, hs2],
                                pk[2 * half + c][0:64, :],
                                b_sb[0:64, c : c + 1],
                            )
                            nc.vector.tensor_scalar_add(
                                qtz[2 * c + 1][64:128, hs2],
                                pk[2 * half + c][64:128, :],
                                b_sb[64:128, c : c + 1],
                            )

            # --- v projection: natural [s, e]; two passes of 8 s-blocks, one
            # [128, 256] accumulator region per PSUM bank (start=True clears
            # the whole bank's has_written bits, so regions must not share).
            # x rides the ACT hwdge queue so the sync queue's kq stream and
            # the vpass stream generate descriptors in parallel. ---
            def emit_vpass(vpass):
                pvm = [
                    ps.tile([128, 1024], F32, tag="q4", bufs=4, name=f"pv{_m}")
                    for _m in range(4)
                ]
                for d2 in range(4):
                    xtile = xt_pool.tile([128, 2, S], BF16, tag="xt", bufs=5)
                    nc.scalar.dma_start(
                        out=xtile[:, :, 0:1024],
                        in_=xvt[
                            d2 * 256 : (d2 + 1) * 256,
                            vpass * 1024 : (vpass + 1) * 1024,
                        ].rearrange("(dd p) s -> p dd s", p=128),
                    )
                    for dd in range(2):
                        d = d2 * 2 + dd
                        for m in range(8):
                            nc.tensor.matmul(
                                pvm[m // 2][:, (m % 2) * 512 : (m % 2) * 512 + 256],
                                xtile[:, dd, m * 128 : (m + 1) * 128],
                                wv_sb[:, d, :],
                                start=(d == 0),
                                stop=(d == 7),
                            )
                for m in range(8):
                    nc.vector.tensor_add(
                        v_sb[:, vpass * 8 + m, :, DK : 2 * DK],
                        pvm[m // 2][
                            :, (m % 2) * 512 : (m % 2) * 512 + 256
                        ].rearrange("p (h d) -> p h d", h=HPC),
                        bv_rep.rearrange("p (h d) -> p h d", h=HPC),
                    )

            emit_kq(xkt, wk_sb, bk_sb, kt)
            emit_vpass(0)
            emit_kq(xqt, wq_sb, bq_sb, None)
            emit_vpass(1)

            # wo loads on the Pool queue during attention (doesn't compete
            # with projection-phase x DMAs or the ACT exp stream)
            for ch in range(8):
                nc.gpsimd.dma_start(out=wo_sb[:, ch, :], in_=wo[:, ch, :])

            # --- attention: one head at a time ---
            mrg = {}

            last_av_mm = [None]

            def emit_attn_head(j, h, send):
                """16 k-tile i-loop for one head + drain into send[:, h, :]."""
                av = ps.tile([128, 1024], F32, tag="q4", bufs=4, name=f"av{j}_{h}")
                for i in range(NI):
                    isl = slice(i * 128, (i + 1) * 128)
                    s = ps.tile([128, 1024], F32, tag="q4", bufs=4)
                    for n in range(2):
                        nsl = slice(n * 512, (n + 1) * 512)
                        qsl = slice(j * JW + n * 512, j * JW + (n + 1) * 512)
                        nc.tensor.matmul(
                            s[:, nsl], kt[h // 2][:, isl], qtz[h][:, qsl],
                            start=True, stop=True,
                        )
                    e = exp_pool.tile([128, 1024], BF16, tag="exp")
                    nc.scalar.activation(e, s, mybir.ActivationFunctionType.Exp)
                    st = dict(start=(i == 0), stop=(i == NI - 1))
                    for n in range(2):
                        nsl = slice(n * 512, (n + 1) * 512)
                        last_av_mm[0] = nc.tensor.matmul(
                            av[:, nsl], v_sb[:, i, h, :], e[:, nsl], **st
                        )
                # drain: denominator (psum parts 0:64) -> approx recip;
                # values enter the mul as a shifted PSUM operand.
                rec = rec_pool.tile([64, 1024], F32, tag="rec")
                nc.vector.reciprocal_approx_fast(out=rec, in_=av[0:64, :])
                nc.vector.tensor_mul(send[:, h, :], av[64:128, :], rec)

            def emit_sends_pair(j, p, send, h):
                """Stage head h of pair p into its pair A2A buffer (2 issues)."""
                hh = h % 2
                for g in range(2):
                    nc.sync.dma_start(
                        out=a2a_in[(j, p)][
                            g * 4 : (g + 1) * 4, hh * DK : (hh + 1) * DK, :
                        ].rearrange("r q c -> q r c"),
                        in_=send[:, h, :].rearrange("q (r c) -> q r c", c=256),
                    )

            def emit_sends_head(h, send):
                for g in range(2):
                    nc.sync.dma_start(
                        out=a2a_in_h[h][g * 4 : (g + 1) * 4, :, :].rearrange(
                            "r q c -> q r c"
                        ),
                        in_=send[:, h, :].rearrange("q (r c) -> q r c", c=256),
                    )

            def emit_collective(inb, outb):
                nc.gpsimd.collective_compute(
                    "AllToAll",
                    mybir.AluOpType.bypass,
                    replica_groups=[list(range(N_CORES))],
                    ins=[inb[:]],
                    outs=[outb[:]],
                )

            def emit_a2l_pair(j, p):
                """Batched reload of a pair collective: 1 DMA per half."""
                a2lo = a2l_pool.tile([128, 4, 256], BF16, tag="a2l", bufs=8)
                a2hi = a2l_pool.tile([128, 4, 256], BF16, tag="a2l", bufs=8)
                nc.sync.dma_start(
                    out=a2lo, in_=a2a_out[(j, p)][0:4].rearrange("r q c -> q r c")
                )
                nc.sync.dma_start(
                    out=a2hi, in_=a2a_out[(j, p)][4:8].rearrange("r q c -> q r c")
                )
                return a2lo, a2hi

            def emit_merge_pair(j, p, a2lo, a2hi):
                # on GPSIMD (idle engine): a blocked merge must never sit in
                # the DVE ring ahead of a PE-critical drain (PSUM release)
                tmp = a2l_pool.tile([128, 4, 256], BF16, tag="mrt", bufs=2)
                tmp_lo = a2l_pool.tile([128, 4, 256], BF16, tag="mrt2", bufs=2)
                mg = a2l_pool.tile([128, 4, 256], BF16, tag="mrg", bufs=4)
                nc.gpsimd.tensor_scalar_mul(tmp, a2hi, msel_sb[:, 1:2])
                nc.gpsimd.tensor_scalar_mul(tmp_lo, a2lo, msel_sb[:, 0:1])
                nc.gpsimd.tensor_add(mg, tmp_lo, tmp)
                mrg[(j, p)] = mg

            a2t = {}

            # ---- chunk j=0: two pair collectives ----
            send0 = snd_pool.tile([64, HPC, JW], BF16, tag="send", name="send0")
            for p in range(NP):
                for h in (2 * p, 2 * p + 1):
                    emit_attn_head(0, h, send0)
                    emit_sends_pair(0, p, send0, h)
                emit_collective(a2a_in[(0, p)], a2a_out[(0, p)])
                a2t[(0, p)] = emit_a2l_pair(0, p)
            # wo bias broadcast rides the ACT queue behind j0's exps:
            # transfers during attention when DMA is otherwise idle
            nc.scalar.dma_start(
                out=bo_rep,
                in_=bass.AP(tensor=bo[:].tensor, offset=0, ap=[[0, 128], [1, D]]),
            )

            # ---- chunk j=1: pair collective for p0, per-head for p1 ----
            send1 = snd_pool.tile([64, HPC, JW], BF16, tag="send", name="send1")
            for h in (0, 1):
                emit_attn_head(1, h, send1)
                emit_sends_pair(1, 0, send1, h)
            emit_collective(a2a_in[(1, 0)], a2a_out[(1, 0)])
            a2t[(1, 0)] = emit_a2l_pair(1, 0)
            # j0 merges: after j1p0's drains in DVE order, so they run in the
            # DVE idle window during j1p1's attention with data long since
            # landed, and never gate a PSUM release.
            for p in range(NP):
                emit_merge_pair(0, p, *a2t[(0, p)])

            # final pair: BISECT — pair collective (baseline style)
            # final pair: per-head quarter collectives (tail-critical).
            # Partition-offset SBUF DMA writes (LH[64:128]) are not seen by
            # the dependency tracker, so the merge's edges onto all four LH
            # loads are pinned explicitly with add_dep_helper.
            LH = a2l_pool.tile([128, 2, 4, 256], BF16, tag="lh", bufs=1, name="lh")
            emit_attn_head(1, 2, send1)
            emit_sends_head(2, send1)
            emit_collective(a2a_in_h[2], a2a_out_h[2])
            d_lo2 = nc.sync.dma_start(
                out=LH[0:64, 0, :, :], in_=a2a_out_h[2][0:4].rearrange("r q c -> q r c")
            )
            d_hi2 = nc.sync.dma_start(
                out=LH[0:64, 1, :, :], in_=a2a_out_h[2][4:8].rearrange("r q c -> q r c")
            )
            emit_merge_pair(1, 0, *a2t[(1, 0)])
            emit_attn_head(1, 3, send1)
            emit_sends_head(3, send1)
            emit_collective(a2a_in_h[3], a2a_out_h[3])
            d_lo3 = nc.sync.dma_start(
                out=LH[64:128, 0, :, :],
                in_=a2a_out_h[3][0:4].rearrange("r q c -> q r c"),
            )
            d_hi3 = nc.sync.dma_start(
                out=LH[64:128, 1, :, :],
                in_=a2a_out_h[3][4:8].rearrange("r q c -> q r c"),
            )
            mg2 = a2l_pool.tile([128, 4, 256], BF16, tag="mrg", bufs=4, name="mg2")
            tmp2 = a2l_pool.tile([128, 4, 256], BF16, tag="mrt", bufs=2, name="tmp2")
            m_mul = nc.vector.tensor_scalar_mul(tmp2, LH[:, 1, :, :], msel_sb[:, 1:2])
            m_aff = nc.vector.affine_then_add(
                mg2, LH[:, 0, :, :], tmp2, scale=msel_sb[:, 0:1], bias=0.0
            )
            for m_op, dmas in ((m_mul, (d_hi2, d_hi3)), (m_aff, (d_lo2, d_lo3))):
                for d_op in dmas:
                    bass._add_dep_helper(
                        m_op.ins, d_op.ins, sync=True,
                        reason="partition-offset LH DMA not tracked",
                    )
            mrg[(1, 1)] = mg2

            # --- output projections: p-major so everything not depending on
            # the last collective completes during its flight ---
            for j in range(NJ):
                po = [
                    ps.tile([128, 1024], F32, tag="q4", bufs=4, name=f"po{j}_{_m}")
                    for _m in range(2)
                ]
                for p in range(NP):
                    for i in range(4):
                        gch = 2 * i + p
                        for m in range(2):
                            for n in range(2):
                                nsl = slice(n * 512, (n + 1) * 512)
                                mm = nc.tensor.matmul(
                                    po[m][:, nsl],
                                    mrg[(j, p)][:, i, m * 128 : (m + 1) * 128],
                                    wo_sb[:, gch, nsl],
                                    start=(p == 0 and i == 0),
                                    stop=(p == NP - 1 and i == 3),
                                )
                                if p == 0 and i == 0:
                                    # keep every out-proj chain OUT of the PE
                                    # ring until attention is done: the tail
                                    # then hides the last collective behind
                                    # this work instead of stalling mid-run
                                    bass._add_dep_helper(
                                        mm.ins, last_av_mm[0].ins, sync=True,
                                        reason="out-proj after all attention",
                                    )
                for m in range(2):
                    ob = osb_pool.tile([128, D], F32, tag="ob")
                    nc.vector.tensor_add(ob, po[m], bo_rep)
                    nc.sync.dma_start(out=out[j, m * 128 : (m + 1) * 128, :], in_=ob)

    nc.compile()
    return nc


_NC_CACHE = {}


def _get_nc():
    if "nc" not in _NC_CACHE:
        _NC_CACHE["nc"] = _build_nc()
    return _NC_CACHE["nc"]


def kernel(Q, K, V, W_Q, b_Q, W_K, b_K, W_V, b_V, W_O, b_O, _trace=False):
    Q, K, V = (np.asarray(x, np.float32) for x in (Q, K, V))
    W_Q, W_K, W_V, W_O = (np.asarray(x, np.float32) for x in (W_Q, W_K, W_V, W_O))
    b_Q, b_K, b_V, b_O = (np.asarray(x, np.float32) for x in (b_Q, b_K, b_V, b_O))
    scale = np.float32(1.0 / np.sqrt(DK))

    in_maps = []
    for c in range(N_CORES):
        b, g = c // 4, c % 4
        es = slice(g * EC, (g + 1) * EC)
        in_maps.append(
            {
                "xqt": np.ascontiguousarray(Q[b].T[:, _PERM]).astype(ml_dtypes.bfloat16),
                "xkt": np.ascontiguousarray(K[b].T).astype(ml_dtypes.bfloat16),
                "xvt": np.ascontiguousarray(V[b].T).astype(ml_dtypes.bfloat16),
                "wq": _wlayout(W_Q[:, es] * scale),
                "wk": _wlayout(W_K[:, es]),
                "wv": _wlayout(W_V[:, es]),
                "wo": _wo_global(W_O),
                "msel": np.tile(
                    np.array([[1.0 - b, float(b)]], np.float32), (128, 1)
                ),
                "bq": np.ascontiguousarray(b_Q[es] * scale),
                "bk": np.ascontiguousarray(b_K[es]),
                "bv": np.ascontiguousarray(b_V[es]),
                "bo": b_O,
            }
        )

    nc = _get_nc()
    res = run_bass_kernel_spmd(nc, in_maps, list(range(N_CORES)), trace=_trace)

    full = np.empty((B, S, D), np.float32)
    for c in range(N_CORES):
        b, r = c // 4, c % 4
        chunks = res.results[c]["out"]  # [NJ, 256, D]
        full[b, r * 512 : r * 512 + 256, :] = chunks[0]
        full[b, r * 512 + 256 : (r + 1) * 512, :] = chunks[1]
    if _trace:
        return full, res
    return full


# revision 13
# speedup vs baseline: 1.0864x; 1.0864x over previous
"""Multi-head attention (B=2, S=2048, D=1024, H=16) on 8 Trainium2 NeuronCores.

Sharding: core c handles batch b = c//4 and head group g = c%4 (4 heads, 256
of the 1024 model dims). All matmul operands are bf16 (host pre-casts X and
weights); PSUM accumulation stays fp32.

Per core:
  kT/qT = (X @ W_{K,Q}[:, g])^T  [128, 2048] bf16 (score scale folded into
          W_Q/b_Q on host). q stored per-head zero-padded to K=128 so the
          scores matmul drives the full PE array with uniform full-array
          LDWEIGHTS pipelining.
  v     =  X @ W_V[:, g] stored [k, i, head, 128] bf16 with ALL-ONES cols
          0:64 and value cols 64:128, so each av matmul yields the softmax
          denominator replicated on psum parts 0:64 (base-0 for the custom
          approx reciprocal) AND attn@v on parts 64:128 (TT ops may read
          shifted PSUM operands).
  exp on ScalarE (no max-subtraction; scores are O(1) by construction).
  attention runs one HEAD at a time (not pair-interleaved): only one av
          psum accumulator is held across an i-loop, so the next head's
          score tiles always have free PSUM banks and the drain latency
          never stalls the PE at head boundaries.
  drain = single-pass approx reciprocal + per-head multiply writing bf16
          straight into the A2A send tile.
Combine: Q columns host-permuted so attention chunk j covers half of every
core's output rows; 8-core AllToAlls ship each chunk as soon as it drains:
per head-PAIR for chunks (0,p0) (0,p1) (1,p0), and per HEAD for the final
pair so the tail-critical last collective is quarter-size and the
second-to-last ships ~20us earlier. a2l reloads are single batched DMAs on
the sync HWDGE queue right after each collective lands; the msel
batch-select merges are emitted right after the NEXT drain so the in-order
DVE never blocks PSUM release (they run in DVE idle windows).
Output projection: full-width bf16, p-major over A2A chunks so everything
not depending on the last collective completes during its flight. Host
unpermutes.
"""

import sys

if "/opt/trn_rl_repo" not in sys.path:
    sys.path.insert(0, "/opt/trn_rl_repo")

import ml_dtypes
import numpy as np

import concourse.bass as bass
import concourse.mybir as mybir
import concourse.tile as tile
from concourse import bacc
from concourse.bass_utils import run_bass_kernel_spmd

B, S, D = 2, 2048, 1024
H, DK = 16, 64
N_CORES = 8
HPC = 4  # heads per core
EC = HPC * DK  # 256 local model dims per core
F32 = mybir.dt.float32
BF16 = mybir.dt.bfloat16

NJ = 2  # q-chunks of 1024
JW = S // NJ
NI = S // 128  # k-tiles
NP = HPC // 2  # head pairs

# q-column permutation: perm-block r (256 wide) of chunk j = global rows
# [r*512 + j*256 : r*512 + (j+1)*256], so A2A slot r always carries the rows
# core r outputs, half per j-chunk.
_PERM = np.concatenate(
    [np.arange(r * 512 + j * 256, r * 512 + (j + 1) * 256) for j in range(2) for r in range(4)]
)


def _wlayout(w):
    """[1024, EC] -> [128, 8, EC] bf16 matching the SBUF lhsT tile layout."""
    return np.ascontiguousarray(w.reshape(8, 128, EC).transpose(1, 0, 2)).astype(
        ml_dtypes.bfloat16
    )


def _wo_global(W_O):
    """[128, 8, D]: global W_O row-chunks (same for every core)."""
    out = W_O.reshape(8, 128, D).transpose(1, 0, 2)
    return np.ascontiguousarray(out).astype(ml_dtypes.bfloat16)


def _build_nc():
    nc = bacc.Bacc(None, num_devices=N_CORES, num_swdge_queues=4)

    xqt = nc.dram_tensor("xqt", [D, S], BF16, kind="ExternalInput")
    xkt = nc.dram_tensor("xkt", [D, S], BF16, kind="ExternalInput")
    xvt = nc.dram_tensor("xvt", [D, S], BF16, kind="ExternalInput")
    wq = nc.dram_tensor("wq", [128, 8, EC], BF16, kind="ExternalInput")
    wk = nc.dram_tensor("wk", [128, 8, EC], BF16, kind="ExternalInput")
    wv = nc.dram_tensor("wv", [128, 8, EC], BF16, kind="ExternalInput")
    wo = nc.dram_tensor("wo", [128, 8, D], BF16, kind="ExternalInput")
    msel = nc.dram_tensor("msel", [128, 2], F32, kind="ExternalInput")
    bq = nc.dram_tensor("bq", [EC], F32, kind="ExternalInput")
    bk = nc.dram_tensor("bk", [EC], F32, kind="ExternalInput")
    bv = nc.dram_tensor("bv", [EC], F32, kind="ExternalInput")
    bo = nc.dram_tensor("bo", [D], F32, kind="ExternalInput")

    # A2A buffers: pair-sized for (0,0) (0,1) (1,0); head-sized for the
    # final pair's two heads (tail-critical, ship separately).
    a2a_in = {
        (j, p): nc.dram_tensor(f"a2a_in{j}_{p}", [N_CORES, 128, 256], BF16)
        for (j, p) in [(0, 0), (0, 1), (1, 0)]
    }
    a2a_out = {
        (j, p): nc.dram_tensor(f"a2a_out{j}_{p}", [N_CORES, 128, 256], BF16)
        for (j, p) in [(0, 0), (0, 1), (1, 0)]
    }
    # one backing tensor for the two per-head collectives of the final
    # pair, so the a2l reload can pull BOTH heads' slots as a single
    # full-128-partition DMA (partition-offset SBUF DMA writes are NOT
    # dependency-tracked and race with their readers)
    a2a_in_h = {
        h: nc.dram_tensor(f"a2a_inh{h}", [N_CORES, DK, 256], BF16) for h in (2, 3)
    }
    a2a_out_h = {
        h: nc.dram_tensor(f"a2a_outh{h}", [N_CORES, DK, 256], BF16) for h in (2, 3)
    }
    out = nc.dram_tensor("out", [NJ, 256, D], F32, kind="ExternalOutput")

    with tile.TileContext(nc) as tc:
        with (
            tc.tile_pool(name="res", bufs=1) as res,
            tc.tile_pool(name="xt", bufs=10) as xt_pool,
            tc.tile_pool(name="exp", bufs=6) as exp_pool,
            tc.tile_pool(name="osb", bufs=3) as osb_pool,
            tc.tile_pool(name="rec", bufs=4) as rec_pool,
            tc.tile_pool(name="snd", bufs=2) as snd_pool,
            tc.tile_pool(name="a2l", bufs=16) as a2l_pool,
            tc.tile_pool(name="ps", bufs=1, space="PSUM") as ps,
        ):
            # --- weights / biases on the ACT hwdge queue, consumption order ---
            wq_sb = res.tile([128, 8, EC], BF16, tag="wq")
            wk_sb = res.tile([128, 8, EC], BF16, tag="wk")
            wv_sb = res.tile([128, 8, EC], BF16, tag="wv")
            wo_sb = res.tile([128, 8, D], BF16, tag="wo")
            msel_sb = res.tile([128, 2], F32, tag="msel")
            bq_sb = res.tile([128, 2], F32, tag="bq")
            bk_sb = res.tile([128, 2], F32, tag="bk")
            bv_rep = res.tile([128, EC], F32, tag="bv")
            bo_rep = res.tile([128, D], F32, tag="bo")
            nc.scalar.dma_start(out=wk_sb, in_=wk[:])
            nc.scalar.dma_start(out=bk_sb, in_=bk[:].rearrange("(c p) -> p c", p=128))
            nc.scalar.dma_start(out=wv_sb, in_=wv[:])
            nc.scalar.dma_start(out=wq_sb, in_=wq[:])
            nc.scalar.dma_start(out=bq_sb, in_=bq[:].rearrange("(c p) -> p c", p=128))
            nc.scalar.dma_start(out=msel_sb, in_=msel[:])
            nc.scalar.dma_start(
                out=bv_rep,
                in_=bass.AP(tensor=bv[:].tensor, offset=0, ap=[[0, 128], [1, EC]]),
            )

            # --- HAM warm-up: ~3.4us of dummy matmuls so the PE clock
            # gate opens before the first projection data lands ---
            warm = res.tile([128, 512], BF16, tag="warm")
            nc.gpsimd.memset(warm, 0.0)
            pw = ps.tile([128, 512], F32, tag="q4", bufs=4, name="pw")
            for w in range(11):
                nc.tensor.matmul(
                    pw, warm[:, 0:128], warm, start=(w == 0), stop=(w == 10)
                )

            # --- residents ---
            kt = [res.tile([128, S], BF16, tag=f"kt{c}", name=f"kt{c}") for c in range(2)]
            # per-head q, zero-padded in the complementary 64 partitions so the
            # scores matmul contracts K=128 (uniform full-array matmuls)
            qtz = [
                res.tile([128, S], BF16, tag=f"qtz{h}", name=f"qtz{h}")
                for h in range(HPC)
            ]
            for h in range(HPC):
                z = slice(64, 128) if h % 2 == 0 else slice(0, 64)
                nc.gpsimd.memset(qtz[h][z, :], 0.0)
            # v: ones cols 0:64 (denominator), value cols 64:128
            v_sb = res.tile([128, NI, HPC, 2 * DK], BF16, tag="v")
            nc.gpsimd.memset(v_sb[:, :, :, 0:DK], 1.0)

            # --- projections, interleaved k -> v-pass0 -> q -> v-pass1 so
            # the v matmuls fill the PE while xq is still streaming in ---
            # out[e, s] accumulated over d; lhsT = W d-chunk, rhs = X^T.
            def emit_kq(xsrc, w_sb, b_sb, dst):
                pk = [
                    ps.tile([128, 1024], F32, tag="q4", bufs=4, name=f"pk{_c}")
                    for _c in range(4)
                ]
                for d2 in range(4):
                    xtile = xt_pool.tile([128, 2, S], BF16, tag="xt", bufs=5)
                    nc.sync.dma_start(
                        out=xtile,
                        in_=xsrc[d2 * 256 : (d2 + 1) * 256, :].rearrange(
                            "(dd p) s -> p dd s", p=128
                        ),
                    )
                    for dd in range(2):
                        d = d2 * 2 + dd
                        for half in range(2):
                            for c in range(2):
                                for n in range(2):
                                    nc.tensor.matmul(
                                        pk[2 * half + c][:, n * 512 : (n + 1) * 512],
                                        w_sb[:, d, c * 128 : (c + 1) * 128],
                                        xtile[
                                            :,
                                            dd,
                                            half * 1024 + n * 512 : half * 1024
                                            + (n + 1) * 512,
                                        ],
                                        start=(d == 0),
                                        stop=(d == 7),
                                    )
                for half in range(2):
                    hs2 = slice(half * 1024, (half + 1) * 1024)
                    for c in range(2):
                        if dst is not None:
                            nc.vector.tensor_scalar_add(
                                dst[c][:, hs2], pk[2 * half + c], b_sb[:, c : c + 1]
                            )
                        else:
                            nc.vector.tensor_scalar_add(
                                qtz[2 * c][0:64, hs2],
                                pk[2 * half + c][0:64, :],
                                b_sb[0:64, c : c + 1],
                            )
                            nc.vector.tensor_scalar_add(
                                qtz[2 * c + 1][64:128, hs2],
                                pk[2 * half + c][64:128, :],
                                b_sb[64:128, c : c + 1],
                            )

            # --- v projection: natural [s, e]; two passes of 8 s-blocks, one
            # [128, 256] accumulator region per PSUM bank (start=True clears
            # the whole bank's has_written bits, so regions must not share).
            # x rides the ACT hwdge queue so the sync queue's kq stream and
            # the vpass stream generate descriptors in parallel. ---
            def emit_vpass(vpass):
                pvm = [
                    ps.tile([128, 1024], F32, tag="q4", bufs=4, name=f"pv{_m}")
                    for _m in range(4)
                ]
                for d2 in range(4):
                    xtile = xt_pool.tile([128, 2, S], BF16, tag="xt", bufs=5)
                    nc.scalar.dma_start(
                        out=xtile[:, :, 0:1024],
                        in_=xvt[
                            d2 * 256 : (d2 + 1) * 256,
                            vpass * 1024 : (vpass + 1) * 1024,
                        ].rearrange("(dd p) s -> p dd s", p=128),
                    )
                    for dd in range(2):
                        d = d2 * 2 + dd
                        for m in range(8):
                            nc.tensor.matmul(
                                pvm[m // 2][:, (m % 2) * 512 : (m % 2) * 512 + 256],
                                xtile[:, dd, m * 128 : (m + 1) * 128],
                                wv_sb[:, d, :],
                                start=(d == 0),
                                stop=(d == 7),
                            )
                for m in range(8):
                    nc.vector.tensor_add(
                        v_sb[:, vpass * 8 + m, :, DK : 2 * DK],
                        pvm[m // 2][
                            :, (m % 2) * 512 : (m % 2) * 512 + 256
                        ].rearrange("p (h d) -> p h d", h=HPC),
                        bv_rep.rearrange("p (h d) -> p h d", h=HPC),
                    )

            emit_kq(xkt, wk_sb, bk_sb, kt)
            emit_vpass(0)
            emit_kq(xqt, wq_sb, bq_sb, None)
            emit_vpass(1)

            # wo loads on the Pool queue during attention (doesn't compete
            # with projection-phase x DMAs or the ACT exp stream)
            for ch in range(8):
                nc.gpsimd.dma_start(out=wo_sb[:, ch, :], in_=wo[:, ch, :])

            # --- attention: one head at a time ---
            mrg = {}

            last_av_mm = [None]
            drain_mul = {}

            def emit_attn_head(j, h, send):
                """16 k-tile i-loop for one head + drain into send[:, h, :]."""
                av = ps.tile([128, 1024], F32, tag="q4", bufs=4, name=f"av{j}_{h}")
                for i in range(NI):
                    isl = slice(i * 128, (i + 1) * 128)
                    s = ps.tile([128, 1024], F32, tag="q4", bufs=4)
                    for n in range(2):
                        nsl = slice(n * 512, (n + 1) * 512)
                        qsl = slice(j * JW + n * 512, j * JW + (n + 1) * 512)
                        nc.tensor.matmul(
                            s[:, nsl], kt[h // 2][:, isl], qtz[h][:, qsl],
                            start=True, stop=True,
                        )
                    e = exp_pool.tile([128, 1024], BF16, tag="exp")
                    nc.scalar.activation(e, s, mybir.ActivationFunctionType.Exp)
                    st = dict(start=(i == 0), stop=(i == NI - 1))
                    for n in range(2):
                        nsl = slice(n * 512, (n + 1) * 512)
                        last_av_mm[0] = nc.tensor.matmul(
                            av[:, nsl], v_sb[:, i, h, :], e[:, nsl], **st
                        )
                # drain: denominator (psum parts 0:64) -> approx recip;
                # values enter the mul as a shifted PSUM operand.
                rec = rec_pool.tile([64, 1024], F32, tag="rec")
                nc.vector.reciprocal_approx_fast(out=rec, in_=av[0:64, :])
                drain_mul[(j, h)] = nc.vector.tensor_mul(
                    send[:, h, :], av[64:128, :], rec
                )

            def emit_sends_pair(j, p, send, h):
                """Stage head h of pair p into its pair A2A buffer (2 issues)."""
                hh = h % 2
                for g in range(2):
                    nc.sync.dma_start(
                        out=a2a_in[(j, p)][
                            g * 4 : (g + 1) * 4, hh * DK : (hh + 1) * DK, :
                        ].rearrange("r q c -> q r c"),
                        in_=send[:, h, :].rearrange("q (r c) -> q r c", c=256),
                    )

            def emit_sends_head(h, send):
                for g in range(2):
                    nc.sync.dma_start(
                        out=a2a_in_h[h][g * 4 : (g + 1) * 4, :, :].rearrange(
                            "r q c -> q r c"
                        ),
                        in_=send[:, h, :].rearrange("q (r c) -> q r c", c=256),
                    )

            def emit_collective(inb, outb):
                nc.gpsimd.collective_compute(
                    "AllToAll",
                    mybir.AluOpType.bypass,
                    replica_groups=[list(range(N_CORES))],
                    ins=[inb[:]],
                    outs=[outb[:]],
                )

            def emit_a2l_pair(j, p):
                """Batched reload of a pair collective: 1 DMA per half."""
                a2lo = a2l_pool.tile([128, 4, 256], BF16, tag="a2l", bufs=8)
                a2hi = a2l_pool.tile([128, 4, 256], BF16, tag="a2l", bufs=8)
                nc.sync.dma_start(
                    out=a2lo, in_=a2a_out[(j, p)][0:4].rearrange("r q c -> q r c")
                )
                nc.sync.dma_start(
                    out=a2hi, in_=a2a_out[(j, p)][4:8].rearrange("r q c -> q r c")
                )
                return a2lo, a2hi

            def emit_merge_pair(j, p, a2lo, a2hi, after=None):
                # DVE, but pinned AFTER the given drain op: a data-blocked
                # merge must never sit in the in-order DVE ring ahead of a
                # PE-critical drain (PSUM release). By each pin point the
                # merge's collective has long landed, so it runs in the DVE
                # idle window right after that drain.
                tmp = a2l_pool.tile([128, 4, 256], BF16, tag="mrt", bufs=2)
                mg = a2l_pool.tile([128, 4, 256], BF16, tag="mrg", bufs=4)
                m1 = nc.vector.tensor_scalar_mul(tmp, a2hi, msel_sb[:, 1:2])
                nc.vector.affine_then_add(
                    mg, a2lo, tmp, scale=msel_sb[:, 0:1], bias=0.0
                )
                if after is not None:
                    bass._add_dep_helper(
                        m1.ins, after.ins, sync=True,
                        reason="merge only after PE-critical drain",
                    )
                mrg[(j, p)] = mg

            a2t = {}

            # ---- chunk j=0: two pair collectives ----
            send0 = snd_pool.tile([64, HPC, JW], BF16, tag="send", name="send0")
            for p in range(NP):
                for h in (2 * p, 2 * p + 1):
                    emit_attn_head(0, h, send0)
                    emit_sends_pair(0, p, send0, h)
                emit_collective(a2a_in[(0, p)], a2a_out[(0, p)])
                a2t[(0, p)] = emit_a2l_pair(0, p)
            # wo bias broadcast rides the ACT queue behind j0's exps:
            # transfers during attention when DMA is otherwise idle
            nc.scalar.dma_start(
                out=bo_rep,
                in_=bass.AP(tensor=bo[:].tensor, offset=0, ap=[[0, 128], [1, D]]),
            )

            # ---- chunk j=1: pair collective for p0, per-head for p1 ----
            send1 = snd_pool.tile([64, HPC, JW], BF16, tag="send", name="send1")
            for h in (0, 1):
                emit_attn_head(1, h, send1)
                emit_sends_pair(1, 0, send1, h)
            emit_collective(a2a_in[(1, 0)], a2a_out[(1, 0)])
            a2t[(1, 0)] = emit_a2l_pair(1, 0)
            # j0 merges: pinned after j1 h0/h1 drains so they run in DVE
            # idle windows with data long since landed, and never gate a
            # PSUM release.
            for p in range(NP):
                emit_merge_pair(0, p, *a2t[(0, p)], after=drain_mul[(1, p)])

            # final pair: BISECT — pair collective (baseline style)
            # final pair: per-head quarter collectives (tail-critical).
            # Partition-offset SBUF DMA writes (LH[64:128]) are not seen by
            # the dependency tracker, so the merge's edges onto all four LH
            # loads are pinned explicitly with add_dep_helper.
            LH = a2l_pool.tile([128, 2, 4, 256], BF16, tag="lh", bufs=1, name="lh")
            emit_attn_head(1, 2, send1)
            emit_sends_head(2, send1)
            emit_collective(a2a_in_h[2], a2a_out_h[2])
            d_lo2 = nc.sync.dma_start(
                out=LH[0:64, 0, :, :], in_=a2a_out_h[2][0:4].rearrange("r q c -> q r c")
            )
            d_hi2 = nc.sync.dma_start(
                out=LH[0:64, 1, :, :], in_=a2a_out_h[2][4:8].rearrange("r q c -> q r c")
            )
            emit_merge_pair(1, 0, *a2t[(1, 0)], after=drain_mul[(1, 2)])
            emit_attn_head(1, 3, send1)
            emit_sends_head(3, send1)
            emit_collective(a2a_in_h[3], a2a_out_h[3])
            d_lo3 = nc.sync.dma_start(
                out=LH[64:128, 0, :, :],
                in_=a2a_out_h[3][0:4].rearrange("r q c -> q r c"),
            )
            d_hi3 = nc.sync.dma_start(
                out=LH[64:128, 1, :, :],
                in_=a2a_out_h[3][4:8].rearrange("r q c -> q r c"),
            )
            mg2 = a2l_pool.tile([128, 4, 256], BF16, tag="mrg", bufs=4, name="mg2")
            tmp2 = a2l_pool.tile([128, 4, 256], BF16, tag="mrt", bufs=2, name="tmp2")
            m_mul = nc.vector.tensor_scalar_mul(tmp2, LH[:, 1, :, :], msel_sb[:, 1:2])
            m_aff = nc.vector.affine_then_add(
                mg2, LH[:, 0, :, :], tmp2, scale=msel_sb[:, 0:1], bias=0.0
            )
            for m_op, dmas in ((m_mul, (d_hi2, d_hi3)), (m_aff, (d_lo2, d_lo3))):
                for d_op in dmas:
                    bass._add_dep_helper(
                        m_op.ins, d_op.ins, sync=True,
                        reason="partition-offset LH DMA not tracked",
                    )
            mrg[(1, 1)] = mg2

            # --- output projections: p-major so everything not depending on
            # the last collective completes during its flight ---
            for j in range(NJ):
                po = [
                    ps.tile([128, 1024], F32, tag="q4", bufs=4, name=f"po{j}_{_m}")
                    for _m in range(2)
                ]
                for p in range(NP):
                    for i in range(4):
                        gch = 2 * i + p
                        for m in range(2):
                            for n in range(2):
                                nsl = slice(n * 512, (n + 1) * 512)
                                mm = nc.tensor.matmul(
                                    po[m][:, nsl],
                                    mrg[(j, p)][:, i, m * 128 : (m + 1) * 128],
                                    wo_sb[:, gch, nsl],
                                    start=(p == 0 and i == 0),
                                    stop=(p == NP - 1 and i == 3),
                                )
                                if p == 0 and i == 0:
                                    # keep every out-proj chain OUT of the PE
                                    # ring until attention is done: the tail
                                    # then hides the last collective behind
                                    # this work instead of stalling mid-run
                                    bass._add_dep_helper(
                                        mm.ins, last_av_mm[0].ins, sync=True,
                                        reason="out-proj after all attention",
                                    )
                for m in range(2):
                    ob = osb_pool.tile([128, D], F32, tag="ob")
                    nc.vector.tensor_add(ob, po[m], bo_rep)
                    nc.sync.dma_start(out=out[j, m * 128 : (m + 1) * 128, :], in_=ob)

    nc.compile()
    return nc


_NC_CACHE = {}


def _get_nc():
    if "nc" not in _NC_CACHE:
        _NC_CACHE["nc"] = _build_nc()
    return _NC_CACHE["nc"]


def kernel(Q, K, V, W_Q, b_Q, W_K, b_K, W_V, b_V, W_O, b_O, _trace=False):
    Q, K, V = (np.asarray(x, np.float32) for x in (Q, K, V))
    W_Q, W_K, W_V, W_O = (np.asarray(x, np.float32) for x in (W_Q, W_K, W_V, W_O))
    b_Q, b_K, b_V, b_O = (np.asarray(x, np.float32) for x in (b_Q, b_K, b_V, b_O))
    scale = np.float32(1.0 / np.sqrt(DK))

    in_maps = []
    for c in range(N_CORES):
        b, g = c // 4, c % 4
        es = slice(g * EC, (g + 1) * EC)
        in_maps.append(
            {
                "xqt": np.ascontiguousarray(Q[b].T[:, _PERM]).astype(ml_dtypes.bfloat16),
                "xkt": np.ascontiguousarray(K[b].T).astype(ml_dtypes.bfloat16),
                "xvt": np.ascontiguousarray(V[b].T).astype(ml_dtypes.bfloat16),
                "wq": _wlayout(W_Q[:, es] * scale),
                "wk": _wlayout(W_K[:, es]),
                "wv": _wlayout(W_V[:, es]),
                "wo": _wo_global(W_O),
                "msel": np.tile(
                    np.array([[1.0 - b, float(b)]], np.float32), (128, 1)
                ),
                "bq": np.ascontiguousarray(b_Q[es] * scale),
                "bk": np.ascontiguousarray(b_K[es]),
                "bv": np.ascontiguousarray(b_V[es]),
                "bo": b_O,
            }
        )

    nc = _get_nc()
    res = run_bass_kernel_spmd(nc, in_maps, list(range(N_CORES)), trace=_trace)

    full = np.empty((B, S, D), np.float32)
    for c in range(N_CORES):
        b, r = c // 4, c % 4
        chunks = res.results[c]["out"]  # [NJ, 256, D]
        full[b, r * 512 : r * 512 + 256, :] = chunks[0]
        full[b, r * 512 + 256 : (r + 1) * 512, :] = chunks[1]
    if _trace:
        return full, res
    return full
